# revision 40
# baseline (speedup 1.0000x reference)
"""Multi-head attention block (B=4, S=2048, D=1024, H=16) on 8 TRN2 cores.

Sharding: data-parallel over batch (4 batches x 2 cores) and tensor-parallel
over heads (8 heads per core).  Each core computes, for its (batch,
head-group): Q^T/K^T (head-dim-major) and V (seq-major) projections, causal
attention (scores transposed: S^T[k,q] = K Q^T, exp without max-subtraction,
row-sum via an appended ones-column in the PV matmul), context, and a partial
output projection with its w_o column slice.  The host sums the two partial
outputs per batch and adds b_o + b_v @ w_o.T (the v-bias passes through
softmax-normalized attention unchanged, so it is a constant row folded into
the host-side bias).

v3 changes vs the 235us baseline:
  * The Q/K/V projections run as fp8e4(DoubleRow) "3-term" compensated
    matmuls: inputs and (16x-scaled) weights are split on the host into an
    fp8 hi part plus an fp8 residual, and the projection accumulates
    Whi*xhi + Whi*xlo + Wlo*xhi in PSUM (the dropped Wlo*xlo term is
    ~0.1% relative).  In the cost model a DoubleRow matmul retires 256
    contraction rows at 0.5 cycles/output-column, so the 3-term projection
    costs 0.75x of its bf16 equivalent at bf16-level accuracy.  The 1/16
    weight scale is folded into the PSUM evacuation (tensor_scalar mult+add
    with the bias).
  * Attention stays bf16 (fp8 scores/PV/out-proj all fail the 2e-2 gate:
    each raw fp8 quantization point costs 2-4e-2 end-to-end).
  * b_v lives on the host (it passes through softmax unchanged), so the
    post-transpose evacuation is a plain bf16 copy and b_q/b_k are the only
    on-device biases (fused into the projection evacuations with the 1/16
    weight re-scale).
  * Pair-3 has no next pair to project, so chunks 2-3 of its own Q/K
    projections are deferred into its first-head slots as PE filler against
    the exp backlog; V-evac, output-copy, and normalize work rides DVE so
    the ACT engine carries only the exp stream.

Pipeline layout: the Q/K projections for pair p+1 are emitted interleaved
with the attention inner loop for pair p; exp batches two k-blocks per ACT
instruction from a 2-bank PSUM tile; the causal mask is a 128x128 triangle
multiply on DVE; PV sub-blocks are scheduled as fillers inside the next
chunk's score stream.  All input loads ride the SP DMA queue in first-use
order; partial outputs ship as bf16 on the ACT queue.

kernel(**inputs) takes full unsharded inputs and returns the full output.
"""

from collections import deque

import numpy as np

import concourse.bass as bass
import concourse.mybir as mybir
import concourse.tile as tile
from concourse import bacc
from concourse.bass_utils import run_bass_kernel_spmd

B, S, D, H = 4, 2048, 1024, 16
DK = D // H            # 64 head dim
P = 128                # partitions
NCORES = 8
HPC = H // 2           # 8 heads per core
DPC = HPC * DK         # 512 projected dims per core
NPAIR = DPC // P       # 4 head-pairs per core
KT = D // P            # 8 contraction tiles for projections
KT2 = KT // 2          # 4 DoubleRow contraction pairs
SC_W = 512             # projection seq chunk width
N_SC = S // SC_W
QC_W = 512             # query chunk width
N_QC = S // QC_W
NQB = QC_W // P        # 4 query sub-blocks per chunk
NKB = S // P           # 16 key blocks
WS = 16.0              # host-side weight pre-scale (undone at evacuation)
F32 = mybir.dt.float32
BF16 = mybir.dt.bfloat16
F8 = mybir.dt.float8e4
DR = mybir.MatmulPerfMode.DoubleRow

_NC_CACHE: dict = {}


def _build_nc(causal: bool, reps: int = 1) -> bass.Bass:
    nc = bacc.Bacc(
        "TRN2",
        debug=False,
        enable_asserts=False,
        target_bir_lowering=False,
        num_devices=NCORES,
    )

    # hi/lo fp8 input and weight splits (weights pre-scaled by WS on host)
    qT = [nc.dram_tensor(f"qT{i}", [D, S], F8, kind="ExternalInput").ap() for i in range(2)]
    kT = [nc.dram_tensor(f"kT{i}", [D, S], F8, kind="ExternalInput").ap() for i in range(2)]
    vT = [nc.dram_tensor(f"vT{i}", [D, S], F8, kind="ExternalInput").ap() for i in range(2)]
    wqT = [nc.dram_tensor(f"wqT{i}", [D, DPC], F8, kind="ExternalInput").ap() for i in range(2)]
    wkT = [nc.dram_tensor(f"wkT{i}", [D, DPC], F8, kind="ExternalInput").ap() for i in range(2)]
    wvT = [nc.dram_tensor(f"wvT{i}", [D, DPC], F8, kind="ExternalInput").ap() for i in range(2)]
    woT = nc.dram_tensor("woT", [DPC, D], BF16, kind="ExternalInput").ap()
    bq = nc.dram_tensor("bq", [DPC], F32, kind="ExternalInput").ap()
    bk = nc.dram_tensor("bk", [DPC], F32, kind="ExternalInput").ap()
    out = nc.dram_tensor("out", [S, D], BF16, kind="ExternalOutput").ap()

    from contextlib import ExitStack

    with tile.TileContext(nc) as tc, ExitStack() as octx:
        if reps > 1:
            octx.enter_context(tc.For_i(0, reps, 1))
        ctx = octx.enter_context(ExitStack())
        singles = ctx.enter_context(tc.tile_pool(name="singles", bufs=1))

        identity = singles.tile([P, P], BF16)
        from concourse.masks import make_identity
        make_identity(nc, identity)

        if causal:
            # tri[k, u] = 1.0 if u >= k else 0.0 — the causal triangle for a
            # diagonal 128x128 score block whose window starts on its own
            # diagonal (q_global = kb*128 + u, k_global = kb*128 + k).
            tri = singles.tile([P, P], BF16)
            nc.gpsimd.memset(tri, 1.0)
            nc.gpsimd.affine_select(
                out=tri,
                in_=tri,
                compare_op=mybir.AluOpType.is_ge,
                fill=0.0,
                base=0,
                channel_multiplier=-1,
                pattern=[[1, P]],
            )

        # Weights/inputs: [P, hl, KT, *] so a DoubleRow matmul can take
        # [:, hl, kt:kt+2, cols] = [128, 2, n] with the kt pair as dim 1.
        w_v_sb = singles.tile([P, 2, KT, DPC], F8)
        wvr = [t.rearrange("(kt p) d -> p kt d", p=P) for t in wvT]
        nc.sync.dma_start(w_v_sb[:, 0, 0:2], wvr[0][:, 0:2])
        nc.sync.dma_start(w_v_sb[:, 1, 0:2], wvr[1][:, 0:2])

        bq_sb = singles.tile([P, NPAIR], F32)
        bk_sb = singles.tile([P, NPAIR], F32)
        w_q_sb = singles.tile([P, 2, KT, DPC], F8)
        w_k_sb = singles.tile([P, 2, KT, DPC], F8)
        qT_sb = singles.tile([P, 2, KT, S], F8)
        kT_sb = singles.tile([P, 2, KT, S], F8)
        woT_sb = singles.tile([P, NPAIR, D], BF16)

        qr = [t.rearrange("(kt p) s -> p kt s", p=P) for t in qT]
        kr = [t.rearrange("(kt p) s -> p kt s", p=P) for t in kT]

        def emit_early_loads():
            # All loads ride the SP queue in strict first-use order.
            nc.sync.dma_start(w_q_sb[:, 0], wqT[0].rearrange("(kt p) d -> p kt d", p=P))
            nc.sync.dma_start(w_q_sb[:, 1], wqT[1].rearrange("(kt p) d -> p kt d", p=P))
            nc.sync.dma_start(bq_sb, bq.rearrange("(pair p) -> p pair", p=P))
            nc.sync.dma_start(bk_sb, bk.rearrange("(pair p) -> p pair", p=P))
            for hl in range(2):
                nc.sync.dma_start(qT_sb[:, hl, :, 0:SC_W], qr[hl][:, :, 0:SC_W])
            nc.sync.dma_start(w_k_sb[:, 0], wkT[0].rearrange("(kt p) d -> p kt d", p=P))
            nc.sync.dma_start(w_k_sb[:, 1], wkT[1].rearrange("(kt p) d -> p kt d", p=P))
            for hl in range(2):
                nc.sync.dma_start(kT_sb[:, hl, :, 0:SC_W], kr[hl][:, :, 0:SC_W])
            csl = slice(SC_W, 2 * SC_W)
            for hl in range(2):
                nc.sync.dma_start(qT_sb[:, hl, :, csl], qr[hl][:, :, csl])
                nc.sync.dma_start(kT_sb[:, hl, :, csl], kr[hl][:, :, csl])

        def emit_late_loads():
            for sc in range(2, N_SC):
                csl = slice(sc * SC_W, (sc + 1) * SC_W)
                for hl in range(2):
                    nc.sync.dma_start(qT_sb[:, hl, :, csl], qr[hl][:, :, csl])
                    nc.sync.dma_start(kT_sb[:, hl, :, csl], kr[hl][:, :, csl])
            nc.sync.dma_start(
                woT_sb, woT.rearrange("(pair p) dm -> p pair dm", p=P))

        # Persistent activations
        QT_cur = singles.tile([P, 2, S], BF16)   # [d%64(hp), buf, s]
        KT_cur = singles.tile([P, 2, S], BF16)
        V_all = singles.tile([P, NKB, HPC, DK + 1], BF16)  # [s%128, kb, h, d|1]
        ctxT_bf = singles.tile([P, NPAIR, S], BF16)
        nc.vector.memset(V_all[:, :, :, DK:DK + 1], 1.0)

        # (whi,xhi), (whi,xlo), (wlo,xhi) — main term first so the first
        # chunk can start before the residual DMAs land.
        TERMS = ((0, 0), (0, 1), (1, 0))
        NT = len(TERMS) * KT2

        with (
            tc.tile_pool(name="vstage", bufs=2) as vst,
            tc.tile_pool(name="ptpool", bufs=17) as ptp,
            tc.tile_pool(name="little", bufs=8) as lit,
            tc.tile_pool(name="stage", bufs=2) as stg,
            tc.tile_pool(name="ostage", bufs=4) as ost,
            tc.tile_pool(name="qkpsum", bufs=2, space="PSUM") as qkp,
            tc.tile_pool(name="spsum", bufs=2, space="PSUM") as sp,
            tc.tile_pool(name="cpsum", bufs=2, space="PSUM") as cp,
        ):
            def dma_v_x(sc, split=False):
                v_x = vst.tile([P, 2, KT, SC_W], F8, name="v_x")
                vr = [t.rearrange("(kt p) s -> p kt s", p=P)[
                    :, :, sc * SC_W:(sc + 1) * SC_W] for t in vT]
                if split:
                    nc.sync.dma_start(v_x[:, 0, 0:2], vr[0][:, 0:2])
                    nc.sync.dma_start(w_v_sb[:, 0, 2:KT], wvr[0][:, 2:KT])
                    nc.sync.dma_start(v_x[:, 0, 2:KT], vr[0][:, 2:KT])
                    nc.sync.dma_start(v_x[:, 1], vr[1])
                    nc.sync.dma_start(w_v_sb[:, 1, 2:KT], wvr[1][:, 2:KT])
                else:
                    nc.sync.dma_start(v_x[:, 0], vr[0])
                    nc.sync.dma_start(v_x[:, 1], vr[1])
                return v_x

            def emit_v_chunk(sc, v_x):
                for ss in range(SC_W // P):
                    ps = qkp.tile([P, DPC], F32, name="ps_v", tag="ps_qk")
                    n = 0
                    for (wi, xi) in TERMS:
                        for k2 in range(KT2):
                            nc.tensor.matmul(
                                ps,
                                lhsT=v_x[:, xi, 2*k2:2*k2+2, ss * P:(ss + 1) * P],
                                rhs=w_v_sb[:, wi, 2*k2:2*k2+2, :],
                                start=(n == 0),
                                stop=(n == NT - 1),
                                perf_mode=DR,
                            )
                            n += 1
                    sblk = sc * (SC_W // P) + ss
                    # undo the 16x weight scale (DVE: ACT carries the exp
                    # stream, which is the second-longest engine)
                    nc.vector.tensor_scalar_mul(
                        V_all[:, sblk, :, 0:DK],
                        ps.rearrange("p (h d) -> p h d", h=HPC),
                        scalar1=1.0 / WS,
                    )

            def emit_proj_chunk(pair, which, sc):
                """Project one 512-seq chunk of Q^T or K^T for `pair`."""
                w_sb = w_q_sb if which == 0 else w_k_sb
                x_sb = qT_sb if which == 0 else kT_sb
                bias = bq_sb if which == 0 else bk_sb
                dest = QT_cur if which == 0 else KT_cur
                ps = qkp.tile([P, SC_W], F32, name="ps_qk", tag="ps_qk")
                n = 0
                for (wi, xi) in TERMS:
                    for k2 in range(KT2):
                        nc.tensor.matmul(
                            ps,
                            lhsT=w_sb[:, wi, 2*k2:2*k2+2, pair * P:(pair + 1) * P],
                            rhs=x_sb[:, xi, 2*k2:2*k2+2, sc * SC_W:(sc + 1) * SC_W],
                            start=(n == 0),
                            stop=(n == NT - 1),
                            perf_mode=DR,
                        )
                        n += 1
                nc.vector.tensor_scalar(
                    out=dest[:, pair % 2, sc * SC_W:(sc + 1) * SC_W],
                    in0=ps,
                    scalar1=1.0 / WS,
                    scalar2=bias[:, pair:pair + 1],
                    op0=mybir.AluOpType.mult,
                    op1=mybir.AluOpType.add,
                )

            # Prologue emission: V chunk 0, then the pair-0 chunk-0
            # projections so attention starts as early as possible; V
            # chunks 1-3 ride as fillers inside the pair-0 score stream.
            v_x0, v_x1 = dma_v_x(0, split=True), dma_v_x(1)
            emit_early_loads()
            emit_v_chunk(0, v_x0)
            v_x2 = dma_v_x(2)
            emit_proj_chunk(0, 0, 0)
            emit_proj_chunk(0, 1, 0)
            v_x3 = dma_v_x(3)
            emit_late_loads()

            # Pair-0 hp0 filler seed: remaining V chunks + pair-0
            # projections, ordered by need (scores(j) needs proj chunk j;
            # PV(j) needs V chunk j).
            slot_seed = {
                0: [lambda: emit_v_chunk(1, v_x1),
                    lambda: emit_proj_chunk(0, 0, 1),
                    lambda: emit_proj_chunk(0, 1, 1)],
                1: [lambda: emit_v_chunk(2, v_x2),
                    lambda: emit_proj_chunk(0, 0, 2),
                    lambda: emit_proj_chunk(0, 1, 2)],
                2: [lambda: emit_v_chunk(3, v_x3),
                    lambda: emit_proj_chunk(0, 0, 3),
                    lambda: emit_proj_chunk(0, 1, 3)],
            }

            # Next-pair projection chunks per slot j.  Pair 2 emits only 4
            # of pair 3's 8 chunks; chunks 2 and 3 of Q and K are deferred
            # into pair 3's own first-head slots, where the PE otherwise
            # idles against the exp backlog (no next pair left to project).
            def proj_count(pair, hp, j):
                if pair == 0:
                    return 0 if hp == 0 else 2
                if pair == 2 and hp == 1:
                    return 0
                return (0, 1, 1, 2)[j]

            NDC = D // 512

            for pair in range(NPAIR):
                pbuf = pair % 2
                last_pair = pair == NPAIR - 1
                ctx_stage = stg.tile([P, NKB, P], BF16, name="ctx_stage")
                slot = 0

                def emit_transpose(sb, ctx_stage=ctx_stage, pair=pair):
                    # PE transpose ctx_stage[q, d] -> tps PSUM -> ctxT_bf.
                    tps = cp.tile([P, P], BF16, name="tps", tag="cps")
                    nc.tensor.transpose(tps, ctx_stage[:, sb, :], identity)
                    nc.vector.tensor_copy(
                        out=ctxT_bf[:, pair, sb * P:(sb + 1) * P], in_=tps)

                def emit_p3(sb, dmc):
                    # Output projection for one 128x512 block; only legal
                    # once all pairs' ctxT at sb are final (pair-3 tail loop).
                    ps = qkp.tile([P, 512], F32, name="ps_o", tag="ps_qk")
                    for pr in range(NPAIR):
                        nc.tensor.matmul(
                            ps,
                            lhsT=ctxT_bf[:, pr, sb * P:(sb + 1) * P],
                            rhs=woT_sb[:, pr, dmc * 512:(dmc + 1) * 512],
                            start=(pr == 0),
                            stop=(pr == NPAIR - 1),
                        )
                    o_sb = ost.tile([P, 512], BF16, name="o_sb")
                    nc.vector.tensor_copy(out=o_sb, in_=ps)
                    nc.sync.dma_start(
                        out[sb * P:(sb + 1) * P, dmc * 512:(dmc + 1) * 512],
                        o_sb,
                    )

                tp_due = deque()

                for hp in range(2):
                    psl = slice(hp * DK, (hp + 1) * DK)
                    tail = last_pair and hp == 1

                    def emit_scores(j, fillers=None, late=None):
                        """Score blocks for chunk j, causally trimmed, exp'd
                        two k-blocks per ACT instruction.  Returns
                        {kb: (pt_tile, base_col, qstart)}."""
                        kb_hi = min(NKB, (j + 1) * NQB) if causal else NKB
                        kb_diag0 = j * NQB if causal else kb_hi
                        tri_by_c = {}
                        blocks = []  # (kb, qstart, width)
                        for kb in range(kb_hi):
                            if causal and kb >= kb_diag0:
                                c = kb - kb_diag0
                                qs = j * QC_W + c * P
                                w = QC_W - c * P
                            else:
                                qs = j * QC_W
                                w = QC_W
                            blocks.append((kb, qs, w))
                        groups = []
                        nd, dg = blocks[:kb_diag0], blocks[kb_diag0:]
                        if dg:
                            groups.append(dg[0:2])
                        groups += [nd[i:i + 2] for i in range(0, len(nd), 2)]
                        if len(dg) > 2:
                            groups.append(dg[2:4])
                        tiles = {}
                        late_at = min(4, len(groups) - 1)
                        for gi, grp in enumerate(groups):
                            if gi == late_at and late is not None:
                                late()
                            if gi >= 1 and fillers:
                                fillers.popleft()()
                            tw = sum(b[2] for b in grp)
                            ps = sp.tile([P, 1024], F32, name="ps_s", tag="ps_s")
                            pt = ptp.tile([P, 1024], BF16, name="pt", tag="pt")
                            col = 0
                            for (kb, qs, w) in grp:
                                nc.tensor.matmul(
                                    ps[:, col:col + w],
                                    lhsT=KT_cur[psl, pbuf, kb * P:(kb + 1) * P],
                                    rhs=QT_cur[psl, pbuf, qs:qs + w],
                                    start=True,
                                    stop=True,
                                )
                                tiles[kb] = (pt, col, qs)
                                col += w
                            nc.scalar.activation(
                                pt[:, 0:tw], ps[:, 0:tw],
                                mybir.ActivationFunctionType.Exp,
                                scale=1.0 / np.sqrt(DK),
                            )
                            # Triangle masks run lazily right before the
                            # first PV sub-block that reads them.
                            for (kb, qs, w) in grp:
                                if causal and kb >= kb_diag0:
                                    c0 = tiles[kb][1]
                                    tri_by_c[kb - kb_diag0] = (
                                        lambda pt=pt, c0=c0: nc.vector.tensor_mul(
                                            pt[:, c0:c0 + P], pt[:, c0:c0 + P], tri
                                        ))
                        return tiles, tri_by_c

                    def emit_pv(j, scored, qqs):
                        tiles, tri_by_c = scored
                        h = pair * 2 + hp
                        for qq in qqs:
                            if causal and qq in tri_by_c:
                                tri_by_c.pop(qq)()
                            qb = j * NQB + qq
                            kmax = (qb + 1) if causal else NKB
                            cps = cp.tile([P, DK + 1], F32, name="cps", tag="cps")
                            for kb in range(kmax):
                                pt, base, qs = tiles[kb]
                                off = base + qb * P - qs
                                nc.tensor.matmul(
                                    cps,
                                    lhsT=pt[:, off:off + P],
                                    rhs=V_all[:, kb, h, :],
                                    start=(kb == 0),
                                    stop=(kb == kmax - 1),
                                )
                            recip = lit.tile([P, 1], F32, name="recip")
                            nc.vector.reciprocal(recip, cps[:, DK:DK + 1])
                            nc.vector.tensor_scalar_mul(
                                ctx_stage[:, qb, psl], cps[:, 0:DK], scalar1=recip
                            )
                            if hp == 1:
                                # transposes lag one sub-block so the PE
                                # doesn't stall on the DVE scale just above
                                if tail:
                                    emit_transpose(qb)
                                else:
                                    while tp_due and tp_due[0] < qb:
                                        emit_transpose(tp_due.popleft())
                                    tp_due.append(qb)

                    def pv_front(j, scored):
                        emit_pv(j, scored, [0, 1])

                    def pv_back(j, scored):
                        # runs the batched recip + scales + transposes for
                        # the whole chunk, so all four sub-blocks' p3 deps
                        # are emitted here
                        emit_pv(j, scored, [2, 3])
                        if tail:
                            for qq in (0, 1):
                                sb = j * NQB + qq
                                for dmc in range(NDC):
                                    pending.append(
                                        lambda sb=sb, dmc=dmc: emit_p3(sb, dmc))
                            for qq in (2, 3):
                                sb = j * NQB + qq
                                for dmc in range(NDC):
                                    emit_p3(sb, dmc)

                    pending = deque()
                    prev = None
                    for j in range(N_QC):
                        if pair == 0 and hp == 0:
                            pending.extend(slot_seed.get(j, []))
                        if pair + 1 < NPAIR:
                            for _ in range(proj_count(pair, hp, j)):
                                w = (pair + 1, slot % 2, slot // 2)
                                pending.append(
                                    lambda w=w: emit_proj_chunk(w[0], w[1], w[2]))
                                slot += 1
                        elif hp == 0 and j < 2:
                            # self-deferred chunk 2/3 projections (see
                            # proj_count): chunk c is consumed by scores(c),
                            # two slots after its emission here
                            for wh in range(2):
                                pending.append(
                                    lambda wh=wh, c=j + 2: emit_proj_chunk(
                                        NPAIR - 1, wh, c))
                        late = None
                        if prev is not None:
                            late = (lambda j=j, s=prev: pv_back(j - 1, s))
                            pending.appendleft(
                                lambda j=j, s=prev: pv_front(j - 1, s))
                        cur = emit_scores(j, pending, late=late)
                        while pending:
                            pending.popleft()()
                        prev = cur
                    pv_front(N_QC - 1, prev)
                    pv_back(N_QC - 1, prev)
                    while pending:
                        pending.popleft()()

                while tp_due:
                    emit_transpose(tp_due.popleft())

    if not nc.is_finalized():
        nc.finalize()
    return nc


def _get_nc(causal: bool, reps: int = 1, **kw) -> bass.Bass:
    key = (causal, reps, tuple(sorted(kw.items())))
    if key not in _NC_CACHE:
        _NC_CACHE[key] = _build_nc(causal, reps, **kw)
    return _NC_CACHE[key]


def _split8(x):
    """fp8 hi/lo split: x ~= hi + lo, both float8_e4m3 (TRN fp8e4)."""
    import ml_dtypes
    f8 = ml_dtypes.float8_e4m3
    x = np.asarray(x, np.float32)
    hi = np.clip(x, -240, 240).astype(f8)
    lo = np.clip(x - hi.astype(np.float32), -240, 240).astype(f8)
    return np.ascontiguousarray(hi), np.ascontiguousarray(lo)


def _make_in_maps(q, k, v, w_q, w_k, w_v, w_o, b_q, b_k):
    import ml_dtypes
    bfnp = ml_dtypes.bfloat16
    in_maps = []
    qb_ = [_split8(q[b].T) for b in range(B)]
    kb_ = [_split8(k[b].T) for b in range(B)]
    vb_ = [_split8(v[b].T) for b in range(B)]
    for c in range(NCORES):
        b, g = divmod(c, 2)
        hsl = slice(g * DPC, (g + 1) * DPC)
        wq_h, wq_l = _split8(w_q[hsl, :].T * WS)
        wk_h, wk_l = _split8(w_k[hsl, :].T * WS)
        wv_h, wv_l = _split8(w_v[hsl, :].T * WS)
        m = {
            "woT": np.ascontiguousarray(w_o[:, hsl].T.astype(bfnp)),
            "bq": np.ascontiguousarray(b_q[hsl]),
            "bk": np.ascontiguousarray(b_k[hsl]),
        }
        for i in range(2):
            m[f"qT{i}"] = qb_[b][i]
            m[f"kT{i}"] = kb_[b][i]
            m[f"vT{i}"] = vb_[b][i]
        m["wqT0"], m["wqT1"] = wq_h, wq_l
        m["wkT0"], m["wkT1"] = wk_h, wk_l
        m["wvT0"], m["wvT1"] = wv_h, wv_l
        in_maps.append(m)
    return in_maps


def kernel(q, k, v, mask, w_q, b_q, w_k, b_k, w_v, b_v, w_o, b_o, **run_kwargs):
    q = np.asarray(q, np.float32)
    k = np.asarray(k, np.float32)
    v = np.asarray(v, np.float32)
    w_q = np.asarray(w_q, np.float32)
    w_k = np.asarray(w_k, np.float32)
    w_v = np.asarray(w_v, np.float32)
    w_o = np.asarray(w_o, np.float32)
    b_q = np.asarray(b_q, np.float32)
    b_k = np.asarray(b_k, np.float32)
    b_v = np.asarray(b_v, np.float32)
    b_o = np.asarray(b_o, np.float32)

    mask_b = np.asarray(mask).reshape(S, S).astype(bool)
    causal = bool(np.array_equal(mask_b, np.tril(np.ones((S, S), bool))))
    if not causal:
        assert mask_b.all(), "only causal or all-ones masks are supported"

    nc = _get_nc(causal)
    in_maps = _make_in_maps(q, k, v, w_q, w_k, w_v, w_o, b_q, b_k)

    res = run_bass_kernel_spmd(nc, in_maps, core_ids=list(range(NCORES)), **run_kwargs)
    outs = [np.asarray(r["out"], dtype=np.float32) for r in res.results]
    # b_v passes through the softmax-normalized attention unchanged, so its
    # output contribution is the constant row b_v @ w_o.T — folded here.
    b_eff = b_o + b_v @ w_o.T
    full = np.stack(
        [outs[2 * b] + outs[2 * b + 1] + b_eff[None, :] for b in range(B)]
    ).astype(np.float32)
    kernel.last_result = res
    return full


kernel.last_result = None
